# revision 1
# baseline (speedup 1.0000x reference)
"""Causal self-attention (B=2, T=4096, C=768, H=12) on 8 trn2 NeuronCores — v2.

Sharding: core c -> batch b = c//4, head group g = c%4 (3 heads per core).
Each core computes QKV projection for its 3 heads, causal attention, and a
partial output projection; host sums the 4 partials per batch and adds b_proj.

v2 changes vs v1 (552us):
  - S^T matmuls (K=64) issue as row-tiled concurrent pairs via tile_position
    (0,0)/(64,0): heads h0/h1 pack the 128x128 PE array together; h2 is
    self-paired against a duplicated Q^T/K^T copy in partitions 64-127 (the
    dup falls out of the QKV matmul by duplicating w_qk columns host-side).
  - PV matmuls widened to M=128 stationary ([V|1|0..] for h0/h2,
    [1|0..|V] for h1) so Y^T lands in the partition range its consumer
    needs (h0 Y at 0-63 l at 64; h1 l at 0, Y at 64-127) and every matmul
    lights the full PE array — half-array work re-throttles the HAM clock
    gate to 1.2GHz, which is where most of v1's time went.
  - softmax normalize: l row hops to partition 0 (DVE lane-locked copy +
    SBUF->SBUF DMA on the gpsimd queue; the custom DVE recip and the GPSIMD
    partition_broadcast only operate from partition base 0), then DVE
    reciprocal_approx_fast + partition_broadcast + one DVE multiply straight
    out of PSUM. Replaces v1's 3.35us serial DVE RECIPROCAL + hi/lo bf16 PE
    broadcast matmuls.
  - output projection: [wp_h0; wp_h1] contracts K=128 in one matmul, wp_h2
    K=64 in a second accumulating matmul.
"""

import os
import sys

import numpy as np

for _p in ("/opt/trn_rl_repo", "/root/.axon_site/_ro/trn_rl_repo"):
    if os.path.isdir(_p) and _p not in sys.path:
        sys.path.insert(0, _p)

import ml_dtypes

import concourse.bacc as bacc
import concourse.bass as bass
import concourse.mybir as mybir
import concourse.tile as tile
from concourse.bass_utils import run_bass_kernel_spmd

B, T, C = 2, 4096, 768
H, HD = 12, 64
NCORES = 8
HPC = 3  # heads per core
P = 128
NBLK = T // 512  # 8 q-blocks of 512
NKT = T // 128  # 32 k-tiles of 128
KC = C // 128  # 6 contraction chunks

F32 = mybir.dt.float32
BF16 = mybir.dt.bfloat16
BF16_NP = ml_dtypes.bfloat16
AF = mybir.ActivationFunctionType

LAG = 3  # PV trails exp by LAG chunks (h2 block)
LAGKT = 6  # h01: PV trails exp by LAGKT k-tiles

_CACHE = {}


def _build_nc():
    nc = bacc.Bacc("TRN2", target_bir_lowering=False, debug=False)

    xt_d = nc.dram_tensor("xt", [C, T], BF16, kind="ExternalInput")
    wqk_d = nc.dram_tensor("wqk", [C, 512], BF16, kind="ExternalInput")
    wv_d = nc.dram_tensor("wv", [C, HPC * HD], BF16, kind="ExternalInput")
    wp_d = nc.dram_tensor("wp", [P, 2, C], BF16, kind="ExternalInput")
    scale_d = nc.dram_tensor("scale_qk", [P, 4], F32, kind="ExternalInput")
    bias_d = nc.dram_tensor("bias_qk", [P, 4], F32, kind="ExternalInput")
    bv_d = nc.dram_tensor("bv", [P, HPC * HD], F32, kind="ExternalInput")
    mask_d = nc.dram_tensor("mask", [P, P], BF16, kind="ExternalInput")
    out_d = nc.dram_tensor("outT", [C, T], F32, kind="ExternalOutput")

    with tile.TileContext(nc) as tc:
        with (
            tc.tile_pool(name="store", bufs=1) as store,
            tc.tile_pool(name="consts", bufs=1) as consts,
            tc.tile_pool(name="pt_pool", bufs=4) as pt_pool,
            tc.tile_pool(name="rsb_pool", bufs=2) as rsb_pool,
            tc.tile_pool(name="rb_pool", bufs=2) as rb_pool,
            tc.tile_pool(name="osb_pool", bufs=3) as osb_pool,
            tc.tile_pool(name="s_psum", bufs=1, space="PSUM") as s_psum,
            tc.tile_pool(name="y_psum", bufs=1, space="PSUM") as y_psum,
            tc.tile_pool(name="m_psum", bufs=2, space="PSUM") as m_psum,
        ):
            # ---- persistent SBUF storage ----
            XT = store.tile([P, KC, T], BF16)
            WQK = store.tile([P, KC, 512], BF16)
            WV = store.tile([P, KC, HPC * HD], BF16)
            WP = store.tile([P, 2, C], BF16)
            QT01 = store.tile([P, T], BF16)  # h0 @0-63, h1 @64-127
            KT01 = store.tile([P, T], BF16)
            QT2 = store.tile([P, T], BF16)  # h2 duplicated in both halves
            KT2 = store.tile([P, T], BF16)
            # V' stationary layouts (M=128):
            #   h0/h2: [V(64) | 1 | 0*63] -> Y at psum 0-63, l at 64
            #   h1:    [1 | 0*63 | V(64)] -> l at psum 0, Y at 64-127
            VN = store.tile([P, NKT, HPC, P], BF16)
            YN01 = store.tile([P, T], BF16)  # h0 rows 0-63, h1 rows 64-127
            YN2 = store.tile([HD, T], BF16)

            scale_qk = consts.tile([P, 4], F32)
            bias_qk = consts.tile([P, 4], F32)
            bvb = consts.tile([P, HPC * HD], F32)
            mask = consts.tile([P, P], BF16)

            # ---- input DMAs: weights/consts first so the first qkv_group
            # only waits for wqk + xt chunk 0, not the whole 6.3MB x^T ----
            nc.sync.dma_start(WQK[:], wqk_d.rearrange("(k p) c -> p k c", p=P))
            nc.sync.dma_start(scale_qk[:], scale_d[:])
            nc.sync.dma_start(bias_qk[:], bias_d[:])
            nc.sync.dma_start(WV[:], wv_d.rearrange("(k p) c -> p k c", p=P))
            nc.sync.dma_start(WP[:], wp_d[:])
            nc.sync.dma_start(bvb[:], bv_d[:])
            nc.sync.dma_start(mask[:], mask_d[:])
            xt_view = xt_d.rearrange("(k p) t -> p k t", p=P)
            for k in range(KC):
                nc.sync.dma_start(
                    XT[:, k, 0:512], xt_view[:, k, 0:512]
                )
            for n in range(1, NBLK):
                nc.sync.dma_start(
                    XT[:, :, n * 512 : (n + 1) * 512],
                    xt_view[:, :, n * 512 : (n + 1) * 512],
                )

            nc.any.memset(VN[:], 0.0)
            nc.any.memset(VN[:, :, 0, HD : HD + 1], 1.0)
            nc.any.memset(VN[:, :, 1, 0:1], 1.0)
            nc.any.memset(VN[:, :, 2, HD : HD + 1], 1.0)

            # ---- work-group builders ----
            def qkv_group(m, n):
                ps = m_psum.tile([P, 512], F32, tag="misc")
                for k in range(KC):
                    nc.tensor.matmul(
                        ps[:],
                        WQK[:, k, m * P : (m + 1) * P],
                        XT[:, k, n * 512 : (n + 1) * 512],
                        start=(k == 0),
                        stop=(k == KC - 1),
                    )
                dst = (QT01, KT01, QT2, KT2)[m]
                nc.vector.tensor_scalar(
                    dst[:, n * 512 : (n + 1) * 512],
                    ps[:],
                    scale_qk[:, m : m + 1],
                    bias_qk[:, m : m + 1],
                    op0=mybir.AluOpType.mult,
                    op1=mybir.AluOpType.add,
                )

            def v_group(mt):
                vp = m_psum.tile([P, HPC * HD], F32, tag="misc")
                for k in range(KC):
                    nc.tensor.matmul(
                        vp[:],
                        XT[:, k, mt * P : (mt + 1) * P],
                        WV[:, k, :],
                        start=(k == 0),
                        stop=(k == KC - 1),
                    )
                vpv = vp[:].rearrange("p (h d) -> p h d", h=HPC)
                bvv = bvb[:].rearrange("p (h d) -> p h d", h=HPC)
                nc.vector.tensor_add(VN[:, mt, 0, 0:HD], vpv[:, 0, :], bvv[:, 0, :])
                nc.vector.tensor_add(VN[:, mt, 1, HD:P], vpv[:, 1, :], bvv[:, 1, :])
                nc.vector.tensor_add(VN[:, mt, 2, 0:HD], vpv[:, 2, :], bvv[:, 2, :])

            def proj_group(m, n):
                ops = m_psum.tile([P, 512], F32, tag="misc")
                nc.tensor.matmul(
                    ops[:],
                    WP[:, 0, m * P : (m + 1) * P],
                    YN01[:, n * 512 : (n + 1) * 512],
                    start=True,
                    stop=False,
                )
                nc.tensor.matmul(
                    ops[:],
                    WP[0:HD, 1, m * P : (m + 1) * P],
                    YN2[:, n * 512 : (n + 1) * 512],
                    start=False,
                    stop=True,
                )
                osb = osb_pool.tile([P, 512], F32)
                nc.vector.tensor_copy(osb[:], ops[:])
                nc.sync.dma_start(
                    out_d[m * P : (m + 1) * P, n * 512 : (n + 1) * 512],
                    osb[:],
                )

            # ---- filler queue ----
            from collections import deque

            filler_q = deque()
            chunk_done = [0]

            def pop_filler(k):
                for _ in range(k):
                    if not filler_q:
                        return
                    n_final, fn = filler_q.popleft()
                    fn()
                    if n_final is not None:
                        chunk_done[0] = max(chunk_done[0], n_final)

            def drain_through_chunk(n):
                while filler_q and chunk_done[0] < n:
                    pop_filler(1)

            deferred = []

            def flush_norms():
                while deferred:
                    deferred.pop(0)()

            def emit_exp(pt, sps, off0, off1):
                # one exp spanning [off0:1024]; on diagonal chunks this also
                # exp's the unwritten psum gap [512:512+off1] — those pt cols
                # are never read by mask or PV, and stale psum exp's to
                # harmless garbage. One instr saves an ACT init + sem wait.
                nc.scalar.activation(pt[:, off0:], sps[:, off0:], AF.Exp)

            def emit_masks(pt, off0, off1, j0, j1, i):
                if j0 >= 4 * i:
                    nc.vector.tensor_mul(
                        pt[:, off0 : off0 + P], pt[:, off0 : off0 + P], mask[:]
                    )
                if j1 >= 4 * i:
                    nc.vector.tensor_mul(
                        pt[:, 512 + off1 : 512 + off1 + P],
                        pt[:, 512 + off1 : 512 + off1 + P],
                        mask[:],
                    )

            # ---- attention: h0+h1 fused block ----
            # S psum pivoted per-k-tile: one [128,1024] tile holds h0's S in
            # bank A (cols 0:512) and h1's in bank B (cols 512:1024), tags
            # alternating per k-tile -> S(j+2) overwrites only after exp(j)
            # read, giving the S->exp->S chain a full k-tile of slack.
            def attn_block01(i):
                yps0 = y_psum.tile([P, 512], F32, tag="y0")
                yps1 = y_psum.tile([P, 512], F32, tag="y1")
                jlast = 4 * i + 3
                pending = []

                def emit_pv(ent):
                    pt, off, j = ent
                    nc.tensor.matmul(
                        yps0[:, off:],
                        VN[:, j, 0, :],
                        pt[:, off:512],
                        start=(j == 0),
                        stop=(j == jlast),
                    )
                    nc.tensor.matmul(
                        yps1[:, off:],
                        VN[:, j, 1, :],
                        pt[:, 512 + off : 1024],
                        start=(j == 0),
                        stop=(j == jlast),
                    )

                for j in range(4 * i + 4):
                    off = max(0, j - 4 * i) * P
                    sps = s_psum.tile(
                        [P, 1024], F32, tag=("s0" if j % 2 == 0 else "s1")
                    )
                    nc.tensor.matmul(
                        sps[:, off:512],
                        KT01[0:HD, j * P : (j + 1) * P],
                        QT01[0:HD, i * 512 + off : (i + 1) * 512],
                        start=True,
                        stop=True,
                        tile_position=(0, 0),
                    )
                    nc.tensor.matmul(
                        sps[:, 512 + off : 1024],
                        KT01[HD:P, j * P : (j + 1) * P],
                        QT01[HD:P, i * 512 + off : (i + 1) * 512],
                        start=True,
                        stop=True,
                        tile_position=(HD, 0),
                    )
                    pt = pt_pool.tile([P, 1024], BF16, tag="pt01", bufs=7)
                    # one exp spans h0 valid, the (unread) gap, and h1 valid
                    nc.scalar.activation(pt[:, off:], sps[:, off:], AF.Exp)
                    if j >= 4 * i:
                        nc.vector.tensor_mul(
                            pt[:, off : off + P], pt[:, off : off + P], mask[:]
                        )
                        nc.vector.tensor_mul(
                            pt[:, 512 + off : 512 + off + P],
                            pt[:, 512 + off : 512 + off + P],
                            mask[:],
                        )
                    pending.append((pt, off, j))
                    if len(pending) > LAGKT:
                        emit_pv(pending.pop(0))
                    if j == 2:
                        flush_norms()
                    if i < 3:
                        if j % 2 == 1:
                            pop_filler(1)
                    elif j % 4 == 3:
                        pop_filler(1)
                pop_filler(1 + (i + 1) // 2)
                while pending:
                    emit_pv(pending.pop(0))
                # normalize h0: Y at yps0[0:64], l at yps0[64]
                #           h1: l at yps1[0],    Y at yps1[64:128]
                # custom DVE recip and partition_broadcast only work from
                # partition base 0, so h0's l row hops PSUM->SBUF (DVE,
                # lane-locked) then SBUF->SBUF DMA down to partition 0.
                ls0 = rsb_pool.tile([P, 512], F32, tag="ls0", bufs=1)
                lr0 = rsb_pool.tile([1, 512], F32, tag="lr0", bufs=1)
                r0 = rsb_pool.tile([1, 512], F32, tag="r0", bufs=1)
                r1 = rsb_pool.tile([P, 512], F32, tag="r1", bufs=1)
                rb0 = rb_pool.tile([P, 512], F32, tag="rb0", bufs=1)
                rb1 = rb_pool.tile([P, 512], F32, tag="rb1", bufs=1)
                nc.vector.tensor_copy(ls0[HD : HD + 1, :], yps0[HD : HD + 1, :])
                nc.gpsimd.dma_start(lr0[0:1, :], ls0[HD : HD + 1, :])
                # h1's l is already at partition 0: full-tile recip straight
                # from psum (garbage rows never read)
                nc.vector.reciprocal_approx_fast(r1[:, :], yps1[:, :])
                nc.gpsimd.partition_broadcast(rb1[:, :], r1[0:1, :])
                nc.vector.tensor_mul(
                    YN01[HD:P, i * 512 : (i + 1) * 512],
                    yps1[HD:P, :],
                    rb1[HD:P, :],
                )

                def _norm_h0(i=i, yps0=yps0, lr0=lr0, r0=r0, rb0=rb0):
                    # DMA-dependent chain: deferred into the next block so it
                    # doesn't head-of-line-block the DVE/GPSIMD FIFOs
                    nc.vector.reciprocal_approx_fast(r0[0:1, :], lr0[0:1, :])
                    nc.gpsimd.partition_broadcast(rb0[:, :], r0[0:1, :])
                    nc.vector.tensor_mul(
                        YN01[0:HD, i * 512 : (i + 1) * 512],
                        yps0[0:HD, :],
                        rb0[0:HD, :],
                    )

                deferred.append(_norm_h0)

            # ---- attention: h2 self-paired block ----
            def attn_block2(i):
                # y1 is freed by h1's fast inline normalize (no DMA hop)
                yps2 = y_psum.tile([P, 512], F32, tag="y1")
                jlast = 4 * i + 3
                pending = []

                def emit_pv(ent):
                    pt, off0, off1, j0, j1 = ent
                    nc.tensor.matmul(
                        yps2[:, off0:],
                        VN[:, j0, 2, :],
                        pt[:, off0:512],
                        start=(j0 == 0),
                        stop=False,
                    )
                    nc.tensor.matmul(
                        yps2[:, off1:],
                        VN[:, j1, 2, :],
                        pt[:, 512 + off1 : 1024],
                        start=False,
                        stop=(j1 == jlast),
                    )

                for c in range(2 * i + 2):
                    j0, j1 = 2 * c, 2 * c + 1
                    off0 = max(0, j0 - 4 * i) * P
                    off1 = max(0, j1 - 4 * i) * P
                    sps = s_psum.tile(
                        [P, 1024], F32, tag=("s0" if c % 2 == 0 else "s1")
                    )
                    # concurrent pair: j0 from rows 0-63, j1 from rows 64-127
                    nc.tensor.matmul(
                        sps[:, off0:512],
                        KT2[0:HD, j0 * P : (j0 + 1) * P],
                        QT2[0:HD, i * 512 + off0 : (i + 1) * 512],
                        start=True,
                        stop=True,
                        tile_position=(0, 0),
                    )
                    nc.tensor.matmul(
                        sps[:, 512 + off1 : 1024],
                        KT2[HD:P, j1 * P : (j1 + 1) * P],
                        QT2[HD:P, i * 512 + off1 : (i + 1) * 512],
                        start=True,
                        stop=True,
                        tile_position=(HD, 0),
                    )
                    pt = pt_pool.tile([P, 1024], BF16, tag="pt2")
                    emit_exp(pt, sps, off0, off1)
                    emit_masks(pt, off0, off1, j0, j1, i)
                    pending.append((pt, off0, off1, j0, j1))
                    if len(pending) > LAG:
                        emit_pv(pending.pop(0))
                    if c == 1:
                        flush_norms()
                    if i < 3:
                        pop_filler(1)
                    elif c % 3 == 2:
                        pop_filler(1)
                pop_filler(1)
                while pending:
                    emit_pv(pending.pop(0))
                ls2 = rsb_pool.tile([P, 512], F32, tag="ls2", bufs=1)
                lr2 = rsb_pool.tile([1, 512], F32, tag="lr2", bufs=1)
                r2 = rsb_pool.tile([1, 512], F32, tag="r2", bufs=1)
                rb2 = rb_pool.tile([P, 512], F32, tag="rb2", bufs=1)
                nc.vector.tensor_copy(ls2[HD : HD + 1, :], yps2[HD : HD + 1, :])
                nc.gpsimd.dma_start(lr2[0:1, :], ls2[HD : HD + 1, :])

                def _norm_h2(i=i, yps2=yps2, lr2=lr2, r2=r2, rb2=rb2):
                    nc.vector.reciprocal_approx_fast(r2[0:1, :], lr2[0:1, :])
                    nc.gpsimd.partition_broadcast(rb2[:, :], r2[0:1, :])
                    nc.vector.tensor_mul(
                        YN2[0:HD, i * 512 : (i + 1) * 512],
                        yps2[0:HD, :],
                        rb2[0:HD, :],
                    )

                deferred.append(_norm_h2)

            # ---- prologue: only chunk 0 dense (the minimum block 0 needs);
            # chunks 1+ become filler supply for the starved early blocks ----
            for m in range(4):
                qkv_group(m, 0)
                v_group(m)

            for n in range(1, NBLK):
                for m in range(4):
                    filler_q.append((None, lambda m=m, n=n: qkv_group(m, n)))
                    filler_q.append(
                        (n if m == 3 else None, lambda t=4 * n + m: v_group(t))
                    )

            # ---- main pipeline ----
            for i in range(NBLK):
                drain_through_chunk(i)
                attn_block01(i)
                pop_filler(1)
                attn_block2(i)
                pop_filler(1)
                for m in range(KC):
                    filler_q.append((None, lambda m=m, n=i: proj_group(m, n)))

            flush_norms()
            while filler_q:
                pop_filler(1)

    nc.compile()
    return nc


def _per_core_inputs(c, x, w_attn, b_attn, xt_cache):
    b, g = divmod(c, 4)
    hs = [HPC * g + j for j in range(HPC)]

    if b not in xt_cache:
        xt_cache[b] = np.ascontiguousarray(x[b].T).astype(BF16_NP)
    xt = xt_cache[b]

    qc = lambda h: w_attn[:, h * HD : (h + 1) * HD]
    kc = lambda h: w_attn[:, C + h * HD : C + (h + 1) * HD]
    vc = lambda h: w_attn[:, 2 * C + h * HD : 2 * C + (h + 1) * HD]
    wqk = np.concatenate(
        [
            qc(hs[0]), qc(hs[1]),
            kc(hs[0]), kc(hs[1]),
            qc(hs[2]), qc(hs[2]),
            kc(hs[2]), kc(hs[2]),
        ],
        axis=1,
    ).astype(BF16_NP)
    wv = np.concatenate([vc(h) for h in hs], axis=1).astype(BF16_NP)

    bq = lambda h: b_attn[h * HD : (h + 1) * HD]
    bk = lambda h: b_attn[C + h * HD : C + (h + 1) * HD]
    sc = 1.0 / np.sqrt(np.float32(HD))
    bias_qk = np.stack(
        [
            np.concatenate([bq(hs[0]), bq(hs[1])]) * sc,
            np.concatenate([bk(hs[0]), bk(hs[1])]),
            np.concatenate([bq(hs[2]), bq(hs[2])]) * sc,
            np.concatenate([bk(hs[2]), bk(hs[2])]),
        ],
        axis=1,
    ).astype(np.float32)
    scale_qk = np.stack(
        [np.full(P, sc), np.ones(P), np.full(P, sc), np.ones(P)],
        axis=1,
    ).astype(np.float32)
    bv = np.broadcast_to(
        np.concatenate(
            [b_attn[2 * C + h * HD : 2 * C + (h + 1) * HD] for h in hs]
        ).astype(np.float32)[None, :],
        (P, HPC * HD),
    ).copy()

    mask = (np.arange(P)[:, None] <= np.arange(P)[None, :]).astype(BF16_NP)

    return {
        "xt": xt,
        "wqk": wqk,
        "wv": wv,
        "wp": None,
        "scale_qk": scale_qk,
        "bias_qk": bias_qk,
        "bv": bv,
        "mask": mask,
    }


def build_in_maps(x, w_attn, b_attn, w_proj):
    x = np.asarray(x, np.float32)
    w_attn = np.asarray(w_attn, np.float32)
    b_attn = np.asarray(b_attn, np.float32)
    w_proj = np.asarray(w_proj, np.float32)

    xt_cache = {}
    in_maps = []
    for c in range(NCORES):
        m = _per_core_inputs(c, x, w_attn, b_attn, xt_cache)
        g = c % 4
        hs = [HPC * g + j for j in range(HPC)]
        wp01 = np.concatenate(
            [
                w_proj[hs[0] * HD : (hs[0] + 1) * HD, :],
                w_proj[hs[1] * HD : (hs[1] + 1) * HD, :],
            ]
        )
        wp2 = np.concatenate(
            [
                w_proj[hs[2] * HD : (hs[2] + 1) * HD, :],
                np.zeros((HD, C), np.float32),
            ]
        )
        m["wp"] = np.stack([wp01, wp2], axis=1).astype(BF16_NP)
        in_maps.append(m)
    return in_maps


def kernel(x, w_attn, b_attn, w_proj, b_proj, _return_raw=False):
    x = np.asarray(x, np.float32)
    b_proj = np.asarray(b_proj, np.float32)

    if "nc" not in _CACHE:
        _CACHE["nc"] = _build_nc()
    nc = _CACHE["nc"]

    in_maps = build_in_maps(x, w_attn, b_attn, w_proj)
    res = run_bass_kernel_spmd(nc, in_maps, list(range(NCORES)))
    outs = [r["outT"] for r in res.results]

    full = np.empty((B, T, C), np.float32)
    for b in range(B):
        acc = outs[4 * b].astype(np.float32).copy()
        for g in range(1, 4):
            acc += outs[4 * b + g]
        full[b] = acc.T
    full += b_proj[None, None, :]
    if _return_raw:
        return full, res
    return full



# revision 2
# speedup vs baseline: 1.0386x; 1.0386x over previous
"""Causal self-attention (B=2, T=4096, C=768, H=12) on 8 trn2 NeuronCores — v3.

Sharding: core c -> batch b = c//4, head group g = c%4 (3 heads per core).

v3 changes vs v2 (319us):
  - All heads self-paired: each S step computes k-tile pair (j0,j1) of ONE
    head concurrently via tile_position (0,0)/(64,0). The partition-duplicated
    Q^T/K^T come from one SBUF->SBUF "swap halves" DMA per qkv m-group block
    (QTa=[q0|q1] natural psum copy, QTb=swap(QTa)=[q1|q0]) instead of
    duplicated weight columns -> QKV shrinks to 3 m-groups.
  - PV in fp8e4 with perf_mode=DoubleRow: one MM contracts both k-tiles of a
    step (V' [128,2,128] stationary, pt [128,2,512] moving) -> halves PV
    stream time. Diagonal steps stay split (2 plain fp8 MMs) to skip the
    invalid above-diagonal columns.
  - exp split across ScalarE and VectorE: ScalarE = ACTIVATE Exp (fp8 out);
    VectorE = single tensor_scalar op computing round(s*8*log2e + 55.55) into
    int8 = the fp8e4m3 BIT PATTERN of exp(s) (Schraudolph in fp8 bits; f32->
    int8 convert is round-to-nearest + saturating on HW, so -30000-masked
    scores land at -128 = -0.0 fp8). Blocks i=0 keep an exact bf16 path
    (ACT exp -> bf16 pt, bf16 V) because early rows have tiny L_eff; i=1 uses
    ACT fp8; i>=2 steps route by a fractional accumulator to balance engines.
  - Causal masking via PE: diagonal tiles get M_tri (upper=-30000) added in
    PSUM by an identity-weight matmul appended to the S accumulation group;
    no DVE mask multiplies remain.
  - l (softmax denom) via V' col of 8.0s: h1/h2 layouts [8|V] put l*8 at psum
    partition 0 -> direct DVE reciprocal_approx_fast from PSUM; h0 keeps
    [V|8] + partition-0 hop (DMA) + deferred norm. V scaled x8 into fp8 to
    dodge e4m3 subnormals; the 8s cancel in Y*(1/(8l))*8.
  - q/k psum->SBUF copies on ScalarE (Identity + per-partition bias), proj
    psum->SBUF on ScalarE (Copy, bf16 out); output DMA'd as bf16.
"""

import os
import sys

import numpy as np

for _p in ("/opt/trn_rl_repo", "/root/.axon_site/_ro/trn_rl_repo"):
    if os.path.isdir(_p) and _p not in sys.path:
        sys.path.insert(0, _p)

import ml_dtypes

import concourse.bacc as bacc
import concourse.bass as bass
import concourse.mybir as mybir
import concourse.tile as tile
from concourse.bass_utils import run_bass_kernel_spmd

B, T, C = 2, 4096, 768
H, HD = 12, 64
NCORES = 8
HPC = 3
P = 128
NBLK = T // 512
NKT = T // 128
NPAIR = NKT // 2
KC = C // 128

F32 = mybir.dt.float32
BF16 = mybir.dt.bfloat16
FP8 = mybir.dt.float8e4
I8 = mybir.dt.int8
BF16_NP = ml_dtypes.bfloat16
FP8_NP = ml_dtypes.float8_e4m3fn
AF = mybir.ActivationFunctionType
ALU = mybir.AluOpType
DR = mybir.MatmulPerfMode.DoubleRow

LOG2E = 1.4426950408889634
A8 = 8.0 * LOG2E
B8 = 55.55

LAG = 5          # PV trails exp by LAG steps
DVE_SHARE = 0.54  # fraction of i>=2 exp steps routed to VectorE

MASKVAL = -30000.0

_CACHE = {}


def _build_nc():
    nc = bacc.Bacc("TRN2", target_bir_lowering=False, debug=False)

    xt_d = nc.dram_tensor("xt", [C, T], BF16, kind="ExternalInput")
    wqk_d = nc.dram_tensor("wqk", [C, 3 * P], BF16, kind="ExternalInput")
    wv_d = nc.dram_tensor("wv", [C, HPC * HD], BF16, kind="ExternalInput")
    wp_d = nc.dram_tensor("wp", [P, 2, C], BF16, kind="ExternalInput")
    bias_d = nc.dram_tensor("bias_qk", [P, 3], F32, kind="ExternalInput")
    bv_d = nc.dram_tensor("bv8", [P, HPC * HD], F32, kind="ExternalInput")
    mtri_d = nc.dram_tensor("mtri", [P, P], BF16, kind="ExternalInput")
    idm_d = nc.dram_tensor("idm", [P, P], BF16, kind="ExternalInput")
    out_d = nc.dram_tensor("outT", [C, T], BF16, kind="ExternalOutput")

    with tile.TileContext(nc) as tc:
        with (
            tc.tile_pool(name="store", bufs=1) as store,
            tc.tile_pool(name="consts", bufs=1) as consts,
            tc.tile_pool(name="pt8_pool", bufs=8) as pt8_pool,
            tc.tile_pool(name="ptb_pool", bufs=3) as ptb_pool,
            tc.tile_pool(name="rsb_pool", bufs=2) as rsb_pool,
            tc.tile_pool(name="rb_pool", bufs=2) as rb_pool,
            tc.tile_pool(name="osb_pool", bufs=3) as osb_pool,
            tc.tile_pool(name="s_psum", bufs=1, space="PSUM") as s_psum,
            tc.tile_pool(name="y_psum", bufs=1, space="PSUM") as y_psum,
            tc.tile_pool(name="m_psum", bufs=2, space="PSUM") as m_psum,
        ):
            # ---- persistent SBUF ----
            XT = store.tile([P, KC, T], BF16)
            WQK = store.tile([P, KC, 3 * P], BF16)
            WV = store.tile([P, KC, HPC * HD], BF16)
            WP = store.tile([P, 2, C], BF16)
            QTa = store.tile([P, T], BF16)
            QTb = store.tile([P, T], BF16)
            KTa = store.tile([P, T], BF16)
            KTb = store.tile([P, T], BF16)
            QKa = store.tile([P, T], BF16)
            QKb = store.tile([P, T], BF16)
            # V' per (pair c, head h, slice s): M-layout
            #   h0: [V(0:64) | 8@64 | 0...]  (Y at psum 0-63, l*8 at 64)
            #   h1/h2: [8@0 | 0 | V(64:128)] (l*8 at psum 0, Y at 64-127)
            VN = store.tile([P, NPAIR, HPC, 2, P], FP8)
            VNB = store.tile([P, 2, HPC, 2, P], BF16)  # pairs 0-1, bf16 for i=0
            YN01 = store.tile([P, T], BF16)  # h0 rows 0-63, h1 rows 64-127
            YN2 = store.tile([P, T], BF16)   # h2 rows 64-127

            bias_qk = consts.tile([P, 3], F32)
            bv8 = consts.tile([P, HPC * HD], F32)
            MTRI = consts.tile([P, P], BF16)
            IDM = consts.tile([P, P], BF16)

            # ---- input DMAs ----
            nc.sync.dma_start(WQK[:], wqk_d.rearrange("(k p) c -> p k c", p=P))
            nc.sync.dma_start(bias_qk[:], bias_d[:])
            nc.sync.dma_start(WV[:], wv_d.rearrange("(k p) c -> p k c", p=P))
            nc.sync.dma_start(WP[:], wp_d[:])
            nc.sync.dma_start(bv8[:], bv_d[:])
            nc.sync.dma_start(MTRI[:], mtri_d[:])
            nc.sync.dma_start(IDM[:], idm_d[:])
            xt_view = xt_d.rearrange("(k p) t -> p k t", p=P)
            for k in range(KC):
                nc.sync.dma_start(XT[:, k, 0:512], xt_view[:, k, 0:512])
            for n in range(1, NBLK):
                nc.sync.dma_start(
                    XT[:, :, n * 512 : (n + 1) * 512],
                    xt_view[:, :, n * 512 : (n + 1) * 512],
                )

            nc.any.memset(VN[:], 0.0)
            nc.any.memset(VN[:, :, 0, :, HD : HD + 1], 8.0)
            nc.any.memset(VN[:, :, 1, :, 0:1], 8.0)
            nc.any.memset(VN[:, :, 2, :, 0:1], 8.0)
            nc.any.memset(VNB[:], 0.0)
            nc.any.memset(VNB[:, :, 0, :, HD : HD + 1], 8.0)
            nc.any.memset(VNB[:, :, 1, :, 0:1], 8.0)
            nc.any.memset(VNB[:, :, 2, :, 0:1], 8.0)

            # ---- qkv / v / proj groups ----
            DSTA = (QTa, KTa, QKa)
            DSTB = (QTb, KTb, QKb)

            def qkv_group(m, n):
                ps = m_psum.tile([P, 512], F32, tag="misc")
                for k in range(KC):
                    nc.tensor.matmul(
                        ps[:],
                        WQK[:, k, m * P : (m + 1) * P],
                        XT[:, k, n * 512 : (n + 1) * 512],
                        start=(k == 0),
                        stop=(k == KC - 1),
                    )
                blk = slice(n * 512, (n + 1) * 512)
                da, db = DSTA[m], DSTB[m]
                nc.scalar.activation(
                    da[:, blk], ps[:], AF.Identity, bias=bias_qk[:, m : m + 1]
                )
                nc.sync.dma_start(db[0:HD, blk], da[HD:P, blk])
                nc.sync.dma_start(db[HD:P, blk], da[0:HD, blk])

            def v_group(mt):
                c, s = mt // 2, mt % 2
                vp = m_psum.tile([P, HPC * HD], F32, tag="misc")
                for k in range(KC):
                    nc.tensor.matmul(
                        vp[:],
                        XT[:, k, mt * P : (mt + 1) * P],
                        WV[:, k, :],
                        start=(k == 0),
                        stop=(k == KC - 1),
                    )
                vpv = vp[:].rearrange("p (h d) -> p h d", h=HPC)
                bvv = bv8[:].rearrange("p (h d) -> p h d", h=HPC)
                nc.vector.scalar_tensor_tensor(
                    VN[:, c, 0, s, 0:HD], vp[:, 0:HD], 8.0, bv8[:, 0:HD],
                    op0=ALU.mult, op1=ALU.add,
                )
                nc.vector.scalar_tensor_tensor(
                    VN[:, c, 1:3, s, HD:P], vpv[:, 1:3, :], 8.0, bvv[:, 1:3, :],
                    op0=ALU.mult, op1=ALU.add,
                )
                if mt < 4:
                    nc.vector.scalar_tensor_tensor(
                        VNB[:, c, 0, s, 0:HD], vp[:, 0:HD], 8.0, bv8[:, 0:HD],
                        op0=ALU.mult, op1=ALU.add,
                    )
                    nc.vector.scalar_tensor_tensor(
                        VNB[:, c, 1:3, s, HD:P], vpv[:, 1:3, :], 8.0,
                        bvv[:, 1:3, :], op0=ALU.mult, op1=ALU.add,
                    )

            def proj_group(m, n):
                ops = m_psum.tile([P, 512], F32, tag="misc")
                nc.tensor.matmul(
                    ops[:],
                    WP[:, 0, m * P : (m + 1) * P],
                    YN01[:, n * 512 : (n + 1) * 512],
                    start=True,
                    stop=False,
                )
                nc.tensor.matmul(
                    ops[:],
                    WP[HD:P, 1, m * P : (m + 1) * P],
                    YN2[HD:P, n * 512 : (n + 1) * 512],
                    start=False,
                    stop=True,
                    tile_position=(HD, 0),
                )
                osb = osb_pool.tile([P, 512], BF16)
                nc.scalar.activation(osb[:], ops[:], AF.Copy)
                nc.sync.dma_start(
                    out_d[m * P : (m + 1) * P, n * 512 : (n + 1) * 512],
                    osb[:],
                )

            # ---- filler queue ----
            from collections import deque

            filler_q = deque()
            chunk_done = [0]

            def pop_filler(k):
                for _ in range(k):
                    if not filler_q:
                        return
                    n_final, fn = filler_q.popleft()
                    fn()
                    if n_final is not None:
                        chunk_done[0] = max(chunk_done[0], n_final)

            def drain_through_chunk(n):
                while filler_q and chunk_done[0] < n:
                    pop_filler(1)

            deferred = []

            def flush_norms():
                while deferred:
                    deferred.pop(0)()

            # exp routing accumulator
            route_acc = [0.0]

            def route_dve():
                route_acc[0] += DVE_SHARE
                if route_acc[0] >= 1.0:
                    route_acc[0] -= 1.0
                    return True
                return False

            # S operand tables per head: (KT_lo, KT_hi, QT_lo, QT_hi)
            SOPS = (
                (KTa, KTb, QTa, QTb),
                (KTb, KTa, QTb, QTa),
                (QKb, QKa, QKa, QKb),
            )

            def attn_block(i, h):
                accurate = i == 0
                act_exp = accurate or i == 1
                kt_lo, kt_hi, qt_lo, qt_hi = SOPS[h]
                ytag = "y1" if h == 1 else "y0"
                yps = y_psum.tile([P, 512], F32, tag=ytag)
                vsrc = VNB if accurate else VN
                clast = 2 * i + 1
                pending = []

                def emit_pv(ent):
                    pt, c, off0, off1 = ent
                    if off0 == 0 and off1 == 0 and not accurate:
                        nc.tensor.matmul(
                            yps[:],
                            vsrc[:, c, h, :, :],
                            pt[:].rearrange("p (s n) -> p s n", s=2),
                            start=(c == 0),
                            stop=False,
                            perf_mode=DR,
                        )
                    else:
                        nc.tensor.matmul(
                            yps[:, off0:],
                            vsrc[:, c, h, 0, :],
                            pt[:, off0:512],
                            start=(c == 0),
                            stop=False,
                        )
                        nc.tensor.matmul(
                            yps[:, off1:],
                            vsrc[:, c, h, 1, :],
                            pt[:, 512 + off1 : 1024],
                            start=False,
                            stop=(c == clast),
                        )

                for c in range(2 * i + 2):
                    j0, j1 = 2 * c, 2 * c + 1
                    off0 = max(0, j0 - 4 * i) * P
                    off1 = max(0, j1 - 4 * i) * P
                    sps = s_psum.tile(
                        [P, 1024], F32, tag=("s0" if c % 2 == 0 else "s1")
                    )
                    tri0 = j0 >= 4 * i
                    tri1 = j1 >= 4 * i
                    nc.tensor.matmul(
                        sps[:, off0:512],
                        kt_lo[0:HD, j0 * P : (j0 + 1) * P],
                        qt_lo[0:HD, i * 512 + off0 : (i + 1) * 512],
                        start=True,
                        stop=not tri0,
                        tile_position=(0, 0),
                    )
                    nc.tensor.matmul(
                        sps[:, 512 + off1 : 1024],
                        kt_hi[HD:P, j1 * P : (j1 + 1) * P],
                        qt_hi[HD:P, i * 512 + off1 : (i + 1) * 512],
                        start=True,
                        stop=not tri1,
                        tile_position=(HD, 0),
                    )
                    if tri0:
                        nc.tensor.matmul(
                            sps[:, off0 : off0 + P],
                            IDM[:],
                            MTRI[:],
                            start=False,
                            stop=True,
                            skip_group_check=True,
                        )
                    if tri1:
                        nc.tensor.matmul(
                            sps[:, 512 + off1 : 512 + off1 + P],
                            IDM[:],
                            MTRI[:],
                            start=False,
                            stop=True,
                            skip_group_check=True,
                        )
                    if accurate:
                        pt = ptb_pool.tile([P, 1024], BF16, tag="ptb")
                        nc.scalar.activation(pt[:, off0:], sps[:, off0:], AF.Exp)
                    elif act_exp or not route_dve():
                        pt = pt8_pool.tile([P, 1024], FP8, tag="pt8")
                        nc.scalar.activation(pt[:, off0:], sps[:, off0:], AF.Exp)
                    else:
                        pt = pt8_pool.tile([P, 1024], FP8, tag="pt8")
                        nc.vector.tensor_scalar(
                            pt[:, off0:].bitcast(I8),
                            sps[:, off0:],
                            A8,
                            B8,
                            op0=ALU.mult,
                            op1=ALU.add,
                        )
                    pending.append((pt, c, off0, off1))
                    if len(pending) > LAG:
                        emit_pv(pending.pop(0))
                    if h == 1 and c == 1:
                        flush_norms()
                    if i < 3:
                        if c % 2 == 1:
                            pop_filler(1)
                    elif c % 3 == 2:
                        pop_filler(1)
                pop_filler(1)
                while pending:
                    emit_pv(pending.pop(0))

                # ---- normalize ----
                blk = slice(i * 512, (i + 1) * 512)
                if h == 0:
                    # Y*8 at psum 0-63, l*8 at partition 64: hop + deferred
                    ls = rsb_pool.tile([P, 512], F32, tag="ls0", bufs=1)
                    lr = rsb_pool.tile([1, 512], F32, tag="lr0", bufs=1)
                    r0 = rsb_pool.tile([P, 512], F32, tag="r0", bufs=1)
                    rb0 = rb_pool.tile([P, 512], F32, tag="rb0", bufs=1)
                    nc.scalar.activation(
                        ls[HD : HD + 1, :], yps[HD : HD + 1, :], AF.Copy
                    )
                    nc.gpsimd.dma_start(lr[0:1, :], ls[HD : HD + 1, :])

                    def _norm_h0(i=i, yps=yps, lr=lr, r0=r0, rb0=rb0, blk=blk):
                        nc.vector.reciprocal_approx_fast(r0[0:1, :], lr[0:1, :])
                        nc.gpsimd.partition_broadcast(rb0[:, :], r0[0:1, :])
                        nc.vector.tensor_mul(
                            YN01[0:HD, blk], yps[0:HD, :], rb0[0:HD, :]
                        )

                    deferred.append(_norm_h0)
                else:
                    # l*8 at psum partition 0: direct
                    rtag = "r1" if h == 1 else "r2"
                    r1 = rsb_pool.tile([P, 512], F32, tag=rtag, bufs=1)
                    rb1 = rb_pool.tile([P, 512], F32, tag="rb" + rtag, bufs=1)
                    nc.vector.reciprocal_approx_fast(r1[:, :], yps[:, :])
                    nc.gpsimd.partition_broadcast(rb1[:, :], r1[0:1, :])
                    dst = YN01 if h == 1 else YN2
                    nc.vector.tensor_mul(
                        dst[HD:P, blk], yps[HD:P, :], rb1[HD:P, :]
                    )

            # ---- prologue: block-0 qkv/v dense ----
            for m in range(3):
                qkv_group(m, 0)
            for mt in range(4):
                v_group(mt)

            for n in range(1, NBLK):
                for m in range(3):
                    filler_q.append((None, lambda m=m, n=n: qkv_group(m, n)))
                for s in range(4):
                    filler_q.append(
                        (
                            n if s == 3 else None,
                            lambda t=4 * n + s: v_group(t),
                        )
                    )

            # ---- main pipeline ----
            for i in range(NBLK):
                drain_through_chunk(i)
                for h in range(HPC):
                    attn_block(i, h)
                    pop_filler(1)
                for m in range(KC):
                    filler_q.append((None, lambda m=m, n=i: proj_group(m, n)))

            flush_norms()
            while filler_q:
                pop_filler(1)

    nc.compile()
    return nc


def _per_core_inputs(c, x, w_attn, b_attn, w_proj, xt_cache):
    b, g = divmod(c, 4)
    hs = [HPC * g + j for j in range(HPC)]

    if b not in xt_cache:
        xt_cache[b] = np.ascontiguousarray(x[b].T).astype(BF16_NP)
    xt = xt_cache[b]

    sc = 1.0 / np.sqrt(np.float32(HD))
    qc = lambda h: w_attn[:, h * HD : (h + 1) * HD] * sc
    kc = lambda h: w_attn[:, C + h * HD : C + (h + 1) * HD]
    # m-groups: [q0|q1], [k0|k1], [q2|k2]
    wqk = np.concatenate(
        [qc(hs[0]), qc(hs[1]), kc(hs[0]), kc(hs[1]), qc(hs[2]), kc(hs[2])],
        axis=1,
    ).astype(BF16_NP)
    wv = np.concatenate(
        [w_attn[:, 2 * C + h * HD : 2 * C + (h + 1) * HD] for h in hs], axis=1
    ).astype(BF16_NP)

    bq = lambda h: b_attn[h * HD : (h + 1) * HD] * sc
    bk = lambda h: b_attn[C + h * HD : C + (h + 1) * HD]
    bias_qk = np.stack(
        [
            np.concatenate([bq(hs[0]), bq(hs[1])]),
            np.concatenate([bk(hs[0]), bk(hs[1])]),
            np.concatenate([bq(hs[2]), bk(hs[2])]),
        ],
        axis=1,
    ).astype(np.float32)
    bv8 = np.broadcast_to(
        8.0
        * np.concatenate(
            [b_attn[2 * C + h * HD : 2 * C + (h + 1) * HD] for h in hs]
        ).astype(np.float32)[None, :],
        (P, HPC * HD),
    ).copy()

    # wp: slot0 = [wp_h0; wp_h1]; slot1 rows 64-127 = wp_h2
    wp0 = np.concatenate(
        [
            w_proj[hs[0] * HD : (hs[0] + 1) * HD, :],
            w_proj[hs[1] * HD : (hs[1] + 1) * HD, :],
        ]
    )
    wp1 = np.concatenate(
        [
            np.zeros((HD, C), np.float32),
            w_proj[hs[2] * HD : (hs[2] + 1) * HD, :],
        ]
    )
    wp = np.stack([wp0, wp1], axis=1).astype(BF16_NP)

    kk = np.arange(P)[:, None]
    qq = np.arange(P)[None, :]
    mtri = np.where(kk <= qq, 0.0, MASKVAL).astype(BF16_NP)
    idm = np.eye(P, dtype=BF16_NP)

    return {
        "xt": xt,
        "wqk": wqk,
        "wv": wv,
        "wp": wp,
        "bias_qk": bias_qk,
        "bv8": bv8,
        "mtri": mtri,
        "idm": idm,
    }


def build_in_maps(x, w_attn, b_attn, w_proj):
    x = np.asarray(x, np.float32)
    w_attn = np.asarray(w_attn, np.float32)
    b_attn = np.asarray(b_attn, np.float32)
    w_proj = np.asarray(w_proj, np.float32)

    xt_cache = {}
    return [
        _per_core_inputs(c, x, w_attn, b_attn, w_proj, xt_cache)
        for c in range(NCORES)
    ]


def kernel(x, w_attn, b_attn, w_proj, b_proj, _return_raw=False):
    x = np.asarray(x, np.float32)
    b_proj = np.asarray(b_proj, np.float32)

    if "nc" not in _CACHE:
        _CACHE["nc"] = _build_nc()
    nc = _CACHE["nc"]

    in_maps = build_in_maps(x, w_attn, b_attn, w_proj)
    res = run_bass_kernel_spmd(nc, in_maps, list(range(NCORES)))
    outs = [r["outT"] for r in res.results]

    full = np.empty((B, T, C), np.float32)
    for b in range(B):
        acc = outs[4 * b].astype(np.float32)
        for g in range(1, 4):
            acc += outs[4 * b + g].astype(np.float32)
        full[b] = acc.T
    full += b_proj[None, None, :]
    if _return_raw:
        return full, res
    return full


# revision 14
# speedup vs baseline: 1.0752x; 1.0352x over previous
"""Causal self-attention (B=2, T=4096, C=768, H=12) on 8 trn2 NeuronCores — v3.

Sharding: core c -> batch b = c//4, head group g = c%4 (3 heads per core).

v3 changes vs v2 (319us):
  - All heads self-paired: each S step computes k-tile pair (j0,j1) of ONE
    head concurrently via tile_position (0,0)/(64,0). The partition-duplicated
    Q^T/K^T come from one SBUF->SBUF "swap halves" DMA per qkv m-group block
    (QTa=[q0|q1] natural psum copy, QTb=swap(QTa)=[q1|q0]) instead of
    duplicated weight columns -> QKV shrinks to 3 m-groups.
  - PV in fp8e4 with perf_mode=DoubleRow: one MM contracts both k-tiles of a
    step (V' [128,2,128] stationary, pt [128,2,512] moving) -> halves PV
    stream time. Diagonal steps stay split (2 plain fp8 MMs) to skip the
    invalid above-diagonal columns.
  - exp split across ScalarE and VectorE: ScalarE = ACTIVATE Exp (fp8 out);
    VectorE = single tensor_scalar op computing round(s*8*log2e + 55.55) into
    int8 = the fp8e4m3 BIT PATTERN of exp(s) (Schraudolph in fp8 bits; f32->
    int8 convert is round-to-nearest + saturating on HW, so -30000-masked
    scores land at -128 = -0.0 fp8). Blocks i=0 keep an exact bf16 path
    (ACT exp -> bf16 pt, bf16 V) because early rows have tiny L_eff; i=1 uses
    ACT fp8; i>=2 steps route by a fractional accumulator to balance engines.
  - Causal masking via PE: diagonal tiles get M_tri (upper=-30000) added in
    PSUM by an identity-weight matmul appended to the S accumulation group;
    no DVE mask multiplies remain.
  - l (softmax denom) via V' col of 8.0s: h1/h2 layouts [8|V] put l*8 at psum
    partition 0 -> direct DVE reciprocal_approx_fast from PSUM; h0 keeps
    [V|8] + partition-0 hop (DMA) + deferred norm. V scaled x8 into fp8 to
    dodge e4m3 subnormals; the 8s cancel in Y*(1/(8l))*8.
  - q/k psum->SBUF copies on ScalarE (Identity + per-partition bias), proj
    psum->SBUF on ScalarE (Copy, bf16 out); output DMA'd as bf16.
"""

import os
import sys

import numpy as np

for _p in ("/opt/trn_rl_repo", "/root/.axon_site/_ro/trn_rl_repo"):
    if os.path.isdir(_p) and _p not in sys.path:
        sys.path.insert(0, _p)

import ml_dtypes

import concourse.bacc as bacc
import concourse.bass as bass
import concourse.mybir as mybir
import concourse.tile as tile
from concourse.bass_utils import run_bass_kernel_spmd

B, T, C = 2, 4096, 768
H, HD = 12, 64
NCORES = 8
HPC = 3
P = 128
NBLK = T // 512
NKT = T // 128
NPAIR = NKT // 2
KC = C // 128

F32 = mybir.dt.float32
BF16 = mybir.dt.bfloat16
FP8 = mybir.dt.float8e4
I8 = mybir.dt.int8
BF16_NP = ml_dtypes.bfloat16
FP8_NP = ml_dtypes.float8_e4m3fn
AF = mybir.ActivationFunctionType
ALU = mybir.AluOpType
DR = mybir.MatmulPerfMode.DoubleRow

LOG2E = 1.4426950408889634
A8 = 8.0 * LOG2E
B8 = 55.55

LAG = 5          # PV trails exp by LAG steps
DVE_SHARE = 0.54  # fraction of i>=2 exp steps routed to VectorE

MASKVAL = -30000.0

_CACHE = {}


def _build_nc():
    nc = bacc.Bacc("TRN2", target_bir_lowering=False, debug=False)

    xt_d = nc.dram_tensor("xt", [C, T], BF16, kind="ExternalInput")
    wqk_d = nc.dram_tensor("wqk", [C, 3 * P], BF16, kind="ExternalInput")
    wv_d = nc.dram_tensor("wv", [C, HPC * HD], BF16, kind="ExternalInput")
    wp_d = nc.dram_tensor("wp", [P, 2, C], BF16, kind="ExternalInput")
    bias_d = nc.dram_tensor("bias_qk", [P, 3], F32, kind="ExternalInput")
    bv_d = nc.dram_tensor("bv8", [P, HPC * HD], F32, kind="ExternalInput")
    mtri_d = nc.dram_tensor("mtri", [P, P], BF16, kind="ExternalInput")
    idm_d = nc.dram_tensor("idm", [P, P], BF16, kind="ExternalInput")
    out_d = nc.dram_tensor("outT", [C, T], BF16, kind="ExternalOutput")

    with tile.TileContext(nc) as tc:
        with (
            tc.tile_pool(name="store", bufs=1) as store,
            tc.tile_pool(name="consts", bufs=1) as consts,
            tc.tile_pool(name="pt8_pool", bufs=8) as pt8_pool,
            tc.tile_pool(name="ptb_pool", bufs=3) as ptb_pool,
            tc.tile_pool(name="rsb_pool", bufs=2) as rsb_pool,
            tc.tile_pool(name="rb_pool", bufs=2) as rb_pool,
            tc.tile_pool(name="osb_pool", bufs=3) as osb_pool,
            tc.tile_pool(name="s_psum", bufs=1, space="PSUM") as s_psum,
            tc.tile_pool(name="y_psum", bufs=1, space="PSUM") as y_psum,
            tc.tile_pool(name="m_psum", bufs=1, space="PSUM") as m_psum,
        ):
            # ---- persistent SBUF ----
            XT = store.tile([P, KC, T], BF16)
            WQK = store.tile([P, KC, 3 * P], BF16)
            WV = store.tile([P, KC, HPC * HD], BF16)
            WP = store.tile([P, 2, C], BF16)
            QTa = store.tile([P, T], BF16)
            QTb = store.tile([P, T], BF16)
            KTa = store.tile([P, T], BF16)
            KTb = store.tile([P, T], BF16)
            QKa = store.tile([P, T], BF16)
            QKb = store.tile([P, T], BF16)
            # V' per (pair c, head h, slice s): M-layout
            #   h0: [V(0:64) | 8@64 | 0...]  (Y at psum 0-63, l*8 at 64)
            #   h1/h2: [8@0 | 0 | V(64:128)] (l*8 at psum 0, Y at 64-127)
            VN = store.tile([P, NPAIR, HPC, 2, P], FP8)
            VNB = store.tile([P, 2, HPC, 2, P], BF16)  # pairs 0-1, bf16 for i=0
            YN01 = store.tile([P, T], BF16)  # h0 rows 0-63, h1 rows 64-127
            YN2 = store.tile([P, T], BF16)   # h2 rows 64-127

            bias_qk = consts.tile([P, 3], F32)
            bv8 = consts.tile([P, HPC * HD], F32)
            MTRI = consts.tile([P, P], BF16)
            IDM = consts.tile([P, P], BF16)

            # ---- input DMAs: block-0 critical path first ----
            nc.sync.dma_start(WQK[:], wqk_d.rearrange("(k p) c -> p k c", p=P))
            nc.sync.dma_start(bias_qk[:], bias_d[:])
            xt_view = xt_d.rearrange("(k p) t -> p k t", p=P)
            for k in range(KC):
                nc.sync.dma_start(XT[:, k, 0:512], xt_view[:, k, 0:512])
            nc.sync.dma_start(WV[:], wv_d.rearrange("(k p) c -> p k c", p=P))
            nc.sync.dma_start(bv8[:], bv_d[:])
            nc.sync.dma_start(MTRI[:], mtri_d[:])
            nc.sync.dma_start(IDM[:], idm_d[:])
            nc.sync.dma_start(WP[:], wp_d[:])
            for n in range(1, NBLK):
                q = nc.gpsimd if n % 2 == 0 else nc.sync
                q.dma_start(
                    XT[:, :, n * 512 : (n + 1) * 512],
                    xt_view[:, :, n * 512 : (n + 1) * 512],
                )

            nc.any.memset(VN[:], 0.0)
            nc.any.memset(VN[:, :, 0, :, HD : HD + 1], 8.0)
            nc.any.memset(VN[:, :, 1, :, 0:1], 8.0)
            nc.any.memset(VN[:, :, 2, :, 0:1], 8.0)
            nc.any.memset(VNB[:], 0.0)
            nc.any.memset(VNB[:, :, 0, :, HD : HD + 1], 8.0)
            nc.any.memset(VNB[:, :, 1, :, 0:1], 8.0)
            nc.any.memset(VNB[:, :, 2, :, 0:1], 8.0)

            # ---- qkv / v / proj groups ----
            DSTA = (QTa, KTa, QKa)
            DSTB = (QTb, KTb, QKb)

            def misc_tile(alt):
                if alt:
                    mt_y = y_psum.tile([P, 512], F32, tag="y0")
                    return mt_y
                mt_m = m_psum.tile([P, 512], F32, tag="misc")
                return mt_m

            def qkv_group(m, n, alt=False):
                ps = misc_tile(alt)
                for k in range(KC):
                    nc.tensor.matmul(
                        ps[:],
                        WQK[:, k, m * P : (m + 1) * P],
                        XT[:, k, n * 512 : (n + 1) * 512],
                        start=(k == 0),
                        stop=(k == KC - 1),
                    )
                blk = slice(n * 512, (n + 1) * 512)
                da, db = DSTA[m], DSTB[m]
                nc.scalar.activation(
                    da[:, blk], ps[:], AF.Identity, bias=bias_qk[:, m : m + 1]
                )
                nc.sync.dma_start(db[0:HD, blk], da[HD:P, blk])
                nc.sync.dma_start(db[HD:P, blk], da[0:HD, blk])

            def v_group(mt, alt=False):
                c, s = mt // 2, mt % 2
                pst = misc_tile(alt)
                for k in range(KC):
                    nc.tensor.matmul(
                        pst[:, 0 : HPC * HD],
                        XT[:, k, mt * P : (mt + 1) * P],
                        WV[:, k, :],
                        start=(k == 0),
                        stop=(k == KC - 1),
                    )
                vpv = pst[:, 0 : HPC * HD].rearrange("p (h d) -> p h d", h=HPC)
                bvv = bv8[:].rearrange("p (h d) -> p h d", h=HPC)
                nc.vector.scalar_tensor_tensor(
                    VN[:, c, 0, s, 0:HD], pst[:, 0:HD], 8.0, bv8[:, 0:HD],
                    op0=ALU.mult, op1=ALU.add,
                )
                nc.vector.scalar_tensor_tensor(
                    VN[:, c, 1:3, s, HD:P], vpv[:, 1:3, :], 8.0, bvv[:, 1:3, :],
                    op0=ALU.mult, op1=ALU.add,
                )
                if mt < 4:
                    nc.vector.scalar_tensor_tensor(
                        VNB[:, c, 0, s, 0:HD], pst[:, 0:HD], 8.0, bv8[:, 0:HD],
                        op0=ALU.mult, op1=ALU.add,
                    )
                    nc.vector.scalar_tensor_tensor(
                        VNB[:, c, 1:3, s, HD:P], vpv[:, 1:3, :], 8.0,
                        bvv[:, 1:3, :], op0=ALU.mult, op1=ALU.add,
                    )

            def proj_group(m, n, alt=False):
                ops = misc_tile(alt)
                nc.tensor.matmul(
                    ops[:],
                    WP[:, 0, m * P : (m + 1) * P],
                    YN01[:, n * 512 : (n + 1) * 512],
                    start=True,
                    stop=False,
                )
                nc.tensor.matmul(
                    ops[:],
                    WP[HD:P, 1, m * P : (m + 1) * P],
                    YN2[HD:P, n * 512 : (n + 1) * 512],
                    start=False,
                    stop=True,
                    tile_position=(HD, 0),
                )
                osb = osb_pool.tile([P, 512], BF16)
                nc.scalar.activation(osb[:], ops[:], AF.Copy)
                nc.sync.dma_start(
                    out_d[m * P : (m + 1) * P, n * 512 : (n + 1) * 512],
                    osb[:],
                )

            # ---- filler queue ----
            from collections import deque

            filler_q = deque()
            chunk_done = [0]

            def pop_filler(k):
                for _ in range(k):
                    if not filler_q:
                        return
                    n_final, fn = filler_q.popleft()
                    fn()
                    if n_final is not None:
                        chunk_done[0] = max(chunk_done[0], n_final)

            def drain_through_chunk(n):
                while filler_q and chunk_done[0] < n:
                    pop_filler(1)

            deferred = []

            def flush_norms():
                while deferred:
                    deferred.pop(0)()

            # exp routing accumulator
            route_acc = [0.0]

            def route_dve():
                route_acc[0] += DVE_SHARE
                if route_acc[0] >= 1.0:
                    route_acc[0] -= 1.0
                    return True
                return False

            # S operand tables per head: (KT_lo, KT_hi, QT_lo, QT_hi)
            SOPS = (
                (KTa, KTb, QTa, QTb),
                (KTb, KTa, QTb, QTa),
                (QKb, QKa, QKa, QKb),
            )

            gstep = [0]

            def attn_block(i, h):
                accurate = i == 0
                act_exp = accurate or i == 1
                kt_lo, kt_hi, qt_lo, qt_hi = SOPS[h]
                yps = y_psum.tile([P, 512], F32, tag="y0")
                vsrc = VNB if accurate else VN
                clast = 2 * i + 1
                pending = []

                def emit_pv(ent):
                    pt, c, off0, off1 = ent
                    if off0 == 0 and off1 == 0 and not accurate:
                        nc.tensor.matmul(
                            yps[:],
                            vsrc[:, c, h, :, :],
                            pt[:].rearrange("p (s n) -> p s n", s=2),
                            start=(c == 0),
                            stop=False,
                            perf_mode=DR,
                        )
                    else:
                        nc.tensor.matmul(
                            yps[:, off0:],
                            vsrc[:, c, h, 0, :],
                            pt[:, off0:512],
                            start=(c == 0),
                            stop=False,
                        )
                        nc.tensor.matmul(
                            yps[:, off1:],
                            vsrc[:, c, h, 1, :],
                            pt[:, 512 + off1 : 1024],
                            start=False,
                            stop=(c == clast),
                        )

                for c in range(2 * i + 2):
                    j0, j1 = 2 * c, 2 * c + 1
                    off0 = max(0, j0 - 4 * i) * P
                    off1 = max(0, j1 - 4 * i) * P
                    sps = s_psum.tile(
                        [P, 1024], F32, tag=f"s{gstep[0] % 3}"
                    )
                    gstep[0] += 1
                    tri0 = j0 >= 4 * i
                    tri1 = j1 >= 4 * i
                    nc.tensor.matmul(
                        sps[:, off0:512],
                        kt_lo[0:HD, j0 * P : (j0 + 1) * P],
                        qt_lo[0:HD, i * 512 + off0 : (i + 1) * 512],
                        start=True,
                        stop=not tri0,
                        tile_position=(0, 0),
                    )
                    nc.tensor.matmul(
                        sps[:, 512 + off1 : 1024],
                        kt_hi[HD:P, j1 * P : (j1 + 1) * P],
                        qt_hi[HD:P, i * 512 + off1 : (i + 1) * 512],
                        start=True,
                        stop=not tri1,
                        tile_position=(HD, 0),
                    )
                    if tri0:
                        nc.tensor.matmul(
                            sps[:, off0 : off0 + P],
                            IDM[:],
                            MTRI[:],
                            start=False,
                            stop=True,
                            skip_group_check=True,
                        )
                    if tri1:
                        nc.tensor.matmul(
                            sps[:, 512 + off1 : 512 + off1 + P],
                            IDM[:],
                            MTRI[:],
                            start=False,
                            stop=True,
                            skip_group_check=True,
                        )
                    if accurate:
                        pt = ptb_pool.tile([P, 1024], BF16, tag="ptb")
                        nc.scalar.activation(pt[:, off0:], sps[:, off0:], AF.Exp)
                    elif act_exp or not route_dve():
                        pt = pt8_pool.tile([P, 1024], FP8, tag="pt8")
                        nc.scalar.activation(pt[:, off0:], sps[:, off0:], AF.Exp)
                    else:
                        pt = pt8_pool.tile([P, 1024], FP8, tag="pt8")
                        nc.vector.tensor_scalar(
                            pt[:, off0:].bitcast(I8),
                            sps[:, off0:],
                            A8,
                            B8,
                            op0=ALU.mult,
                            op1=ALU.add,
                        )
                    pending.append((pt, c, off0, off1))
                    if len(pending) > LAG:
                        emit_pv(pending.pop(0))
                    if h == 1 and c == 1:
                        flush_norms()
                    if i < 3:
                        if c % 2 == 1:
                            pop_filler(1)
                    elif c % 3 == 2:
                        pop_filler(1)
                pop_filler(1)
                while pending:
                    emit_pv(pending.pop(0))

                # ---- normalize ----
                blk = slice(i * 512, (i + 1) * 512)
                if h == 0:
                    # Y*8 at psum 0-63, l*8 at partition 64: hop + deferred
                    ls = rsb_pool.tile([P, 512], F32, tag="ls0", bufs=1)
                    lr = rsb_pool.tile([1, 512], F32, tag="lr0", bufs=1)
                    r0 = rsb_pool.tile([P, 512], F32, tag="r0", bufs=1)
                    rb0 = rb_pool.tile([P, 512], F32, tag="rb0", bufs=1)
                    nc.scalar.activation(
                        ls[HD : HD + 1, :], yps[HD : HD + 1, :], AF.Copy
                    )
                    nc.gpsimd.dma_start(lr[0:1, :], ls[HD : HD + 1, :])

                    def _norm_h0(i=i, yps=yps, lr=lr, r0=r0, rb0=rb0, blk=blk):
                        nc.vector.reciprocal_approx_fast(r0[0:1, :], lr[0:1, :])
                        nc.gpsimd.partition_broadcast(rb0[:, :], r0[0:1, :])
                        nc.vector.tensor_mul(
                            YN01[0:HD, blk], yps[0:HD, :], rb0[0:HD, :]
                        )

                    deferred.append(_norm_h0)
                else:
                    # l*8 at psum partition 0: direct
                    rtag = "r1" if h == 1 else "r2"
                    r1 = rsb_pool.tile([P, 512], F32, tag=rtag, bufs=1)
                    rb1 = rb_pool.tile([P, 512], F32, tag="rb" + rtag, bufs=1)
                    nc.vector.reciprocal_approx_fast(r1[:, :], yps[:, :])
                    nc.gpsimd.partition_broadcast(rb1[:, :], r1[0:1, :])
                    dst = YN01 if h == 1 else YN2
                    nc.vector.tensor_mul(
                        dst[HD:P, blk], yps[HD:P, :], rb1[HD:P, :]
                    )

            # ---- prologue: block-0 qkv/v dense (alternate psum banks) ----
            for m in range(3):
                qkv_group(m, 0, alt=(m % 2 == 1))
            for mt in range(4):
                v_group(mt, alt=(mt % 2 == 0))

            for n in range(1, NBLK):
                for m in range(3):
                    filler_q.append(
                        (None, lambda m=m, n=n, alt=False: qkv_group(m, n, alt))
                    )
                for s in range(4):
                    filler_q.append(
                        (
                            n if s == 3 else None,
                            lambda t=4 * n + s, alt=False: v_group(t, alt),
                        )
                    )

            # ---- main pipeline ----
            for i in range(NBLK):
                drain_through_chunk(i)
                for h in range(HPC):
                    attn_block(i, h)
                    pop_filler(1)
                for m in range(KC):
                    filler_q.append(
                        (None, lambda m=m, n=i, alt=False: proj_group(m, n, alt))
                    )

            flush_norms()
            alt = False
            while filler_q:
                n_final, fn = filler_q.popleft()
                fn(alt=alt)
                alt = not alt

    nc.compile()
    return nc


def _per_core_inputs(c, x, w_attn, b_attn, w_proj, xt_cache):
    b, g = divmod(c, 4)
    hs = [HPC * g + j for j in range(HPC)]

    if b not in xt_cache:
        xt_cache[b] = np.ascontiguousarray(x[b].T).astype(BF16_NP)
    xt = xt_cache[b]

    sc = 1.0 / np.sqrt(np.float32(HD))
    qc = lambda h: w_attn[:, h * HD : (h + 1) * HD] * sc
    kc = lambda h: w_attn[:, C + h * HD : C + (h + 1) * HD]
    # m-groups: [q0|q1], [k0|k1], [q2|k2]
    wqk = np.concatenate(
        [qc(hs[0]), qc(hs[1]), kc(hs[0]), kc(hs[1]), qc(hs[2]), kc(hs[2])],
        axis=1,
    ).astype(BF16_NP)
    wv = np.concatenate(
        [w_attn[:, 2 * C + h * HD : 2 * C + (h + 1) * HD] for h in hs], axis=1
    ).astype(BF16_NP)

    bq = lambda h: b_attn[h * HD : (h + 1) * HD] * sc
    bk = lambda h: b_attn[C + h * HD : C + (h + 1) * HD]
    bias_qk = np.stack(
        [
            np.concatenate([bq(hs[0]), bq(hs[1])]),
            np.concatenate([bk(hs[0]), bk(hs[1])]),
            np.concatenate([bq(hs[2]), bk(hs[2])]),
        ],
        axis=1,
    ).astype(np.float32)
    bv8 = np.broadcast_to(
        8.0
        * np.concatenate(
            [b_attn[2 * C + h * HD : 2 * C + (h + 1) * HD] for h in hs]
        ).astype(np.float32)[None, :],
        (P, HPC * HD),
    ).copy()

    # wp: slot0 = [wp_h0; wp_h1]; slot1 rows 64-127 = wp_h2
    wp0 = np.concatenate(
        [
            w_proj[hs[0] * HD : (hs[0] + 1) * HD, :],
            w_proj[hs[1] * HD : (hs[1] + 1) * HD, :],
        ]
    )
    wp1 = np.concatenate(
        [
            np.zeros((HD, C), np.float32),
            w_proj[hs[2] * HD : (hs[2] + 1) * HD, :],
        ]
    )
    wp = np.stack([wp0, wp1], axis=1).astype(BF16_NP)

    kk = np.arange(P)[:, None]
    qq = np.arange(P)[None, :]
    mtri = np.where(kk <= qq, 0.0, MASKVAL).astype(BF16_NP)
    idm = np.eye(P, dtype=BF16_NP)

    return {
        "xt": xt,
        "wqk": wqk,
        "wv": wv,
        "wp": wp,
        "bias_qk": bias_qk,
        "bv8": bv8,
        "mtri": mtri,
        "idm": idm,
    }


def build_in_maps(x, w_attn, b_attn, w_proj):
    x = np.asarray(x, np.float32)
    w_attn = np.asarray(w_attn, np.float32)
    b_attn = np.asarray(b_attn, np.float32)
    w_proj = np.asarray(w_proj, np.float32)

    xt_cache = {}
    return [
        _per_core_inputs(c, x, w_attn, b_attn, w_proj, xt_cache)
        for c in range(NCORES)
    ]


def kernel(x, w_attn, b_attn, w_proj, b_proj, _return_raw=False):
    x = np.asarray(x, np.float32)
    b_proj = np.asarray(b_proj, np.float32)

    if "nc" not in _CACHE:
        _CACHE["nc"] = _build_nc()
    nc = _CACHE["nc"]

    in_maps = build_in_maps(x, w_attn, b_attn, w_proj)
    res = run_bass_kernel_spmd(nc, in_maps, list(range(NCORES)))
    outs = [r["outT"] for r in res.results]

    full = np.empty((B, T, C), np.float32)
    for b in range(B):
        acc = outs[4 * b].astype(np.float32)
        for g in range(1, 4):
            acc += outs[4 * b + g].astype(np.float32)
        full[b] = acc.T
    full += b_proj[None, None, :]
    if _return_raw:
        return full, res
    return full
